# revision 11
# baseline (speedup 1.0000x reference)
"""Trainium2 Bass kernel for AnatomicalMaskedLinear (block-masked dense layer).

Reference op:
    mask  = kron(adjacency, ones(256, 128))            # (21*256, 21*128)
    y     = x.reshape(B, 21*128) @ (weight*mask).T + bias
    out   = y.reshape(B, 21, 256)

Strategy (v2):
  * Only nonzero (256o x 128k) blocks are shipped/matmul'd (S=nnz blocks).
  * 8 cores = 4 batch quarters x 2 node-row halves; all cores share one
    SPMD graph (same adjacency-derived schedule), only data differs.
  * bf16 operands, 1024-wide moving operand: one matmul per (node, j)
    block into a 2-bank PSUM tile [128, 1024] -> 237 matmuls/core instead
    of 474 (less NX issue overhead, fewer LDWEIGHTS).
  * PE warm-up: HAM clock gate holds the PE at 1.2 GHz until ~3.4us of
    sustained activity. A memset + dummy-matmul burst issued before the
    first real data arrives flips the gate early so the real stream runs
    at 2.4 GHz almost immediately.
  * All loads ride the sync HWDGE ring in first-use order (x is packed
    host-side in first-use block order so bulk triggers are contiguous);
    the first node's blocks use per-block triggers so the first matmul
    fires as soon as ~0.5 MB has landed instead of waiting for a bulk
    transfer. Everything stays resident in SBUF (13 MB total).
  * PSUM evacuation (bias add + f32->bf16 cast, activation Identity with
    per-partition bias) AND the store trigger both run on the scalar
    engine: evac->store needs no cross-engine event, and output traffic
    halves (bf16). Host upcasts to f32.
  * Fewer DMA triggers + fewer cross-engine events shrink the end-of-NEFF
    event-semaphore drain ladder, which counts toward measured exec time.
"""

import os
import numpy as np
import ml_dtypes

NUM_NODES = 21
IN_F = 128
OUT_F = 256
BATCH = 4096
N_CORES = 8
P_BATCH = 4                      # batch ways
B_C = BATCH // P_BATCH           # 1024 batch rows per core
K_TOTAL = NUM_NODES * IN_F       # 2688
O_C = NUM_NODES * 128            # 2688 out rows per core (half of each node)
N_WARM = 9                       # 512-wide dummy matmuls (HAM clock gate)

_CACHE = {}                      # schedule key -> (nc, sched, jorder)


def _node_order(active):
    """Greedy: minimize newly-required x blocks at each step."""
    loaded = set()
    remaining = set(range(NUM_NODES))
    order = []
    while remaining:
        nxt = min(remaining,
                  key=lambda i: (len(set(active[i]) - loaded), len(active[i]), i))
        order.append(nxt)
        loaded |= set(active[nxt])
        remaining.remove(nxt)
    return order


def _build_schedule(adjacency):
    """[(i, [j...], zero_pad)] in greedy node order; >=1 slot per node.

    Within each node, already-loaded x blocks come first (in load order) and
    newly-required blocks last, so a chain's early matmuls never wait on the
    blocks still streaming in for its tail.
    """
    A = np.asarray(adjacency) != 0
    active = {i: [int(j) for j in np.where(A[i])[0]] for i in range(NUM_NODES)}
    order = _node_order(active)
    jorder = []
    seen = set()
    for i in order:
        for j in active[i]:
            if j not in seen:
                seen.add(j)
                jorder.append(j)
    if not jorder:
        jorder = [0]
    pos = {j: p for p, j in enumerate(jorder)}
    sched = []
    loaded = set()
    for i in order:
        js = active[i]
        if js:
            olds = sorted((j for j in js if j in loaded), key=lambda j: pos[j])
            news = sorted((j for j in js if j not in loaded), key=lambda j: pos[j])
            loaded |= set(js)
            sched.append((i, tuple(olds + news), False))
        else:
            sched.append((i, (jorder[0],), True))
    return tuple(sched), tuple(jorder)


def _w_chunks(sched):
    """Group schedule positions into w DMA chunks: per-node for the first
    few nodes, then ~1MB merged chunks."""
    sizes = [len(js) for _, js, _ in sched]
    chunks = []  # (start_slot, n_slots, last_sched_idx)
    k = 0
    s0 = 0
    while k < len(sched):
        if k < 7:
            n = sizes[k]
            last = k
            k += 1
        else:
            n = 0
            last = k
            while k < len(sched) and (n == 0 or n + sizes[k] <= 34):
                n += sizes[k]
                last = k
                k += 1
        chunks.append((s0, n, last))
        s0 += n
    return chunks


def _build_graph(sched, jorder):
    import concourse.tile as tile
    from concourse import bacc, mybir

    S = sum(len(js) for _, js, _ in sched)
    f32 = mybir.dt.float32
    bf16 = mybir.dt.bfloat16
    pos = {j: p for p, j in enumerate(jorder)}
    NJ = len(jorder)

    nc = bacc.Bacc("TRN2", target_bir_lowering=False, debug=False,
                   num_devices=N_CORES)

    xt_d = nc.declare_dram_parameter("xt", [128, NJ * B_C], bf16, isOutput=False)
    wp_d = nc.declare_dram_parameter("wp", [128, S * 128], bf16, isOutput=False)
    bias_d = nc.declare_dram_parameter("biasr", [128, NUM_NODES], f32,
                                       isOutput=False)
    out_d = nc.declare_dram_parameter("out", [O_C, B_C], bf16, isOutput=True)

    # per-node newly needed x blocks (contiguous in jorder by construction)
    new_js = []
    seen = set()
    for i, js, zero in sched:
        cur = [] if zero else [j for j in js if j not in seen]
        seen |= set(cur)
        new_js.append(cur)

    wchunks = _w_chunks(sched)

    with tile.TileContext(nc) as tc:
        with (
            tc.tile_pool(name="const", bufs=1) as constp,
            tc.tile_pool(name="persist", bufs=1) as persist,
            tc.tile_pool(name="psum", bufs=4, space="PSUM") as psump,
            tc.tile_pool(name="outp", bufs=8) as outp,
        ):
            # --- PE warm-up: memset a small bf16 tile, then dummy matmuls
            # (HAM clock gate: PE runs at 1.2 GHz until ~3.4us of sustained
            # activity; dummies must span the whole DMA wait or the gate
            # re-throttles during the idle gap) ---
            warm_sb = constp.tile([128, 512], bf16)
            nc.gpsimd.memset(warm_sb[:], 0.0)
            warm_ps = psump.tile([128, B_C], f32, tag="acc", name="warm")
            for t in range(N_WARM):
                nc.tensor.matmul(warm_ps[:, 0:512], warm_sb[:, 0:128],
                                 warm_sb[:], start=True, stop=True)

            bias_sb = constp.tile([128, NUM_NODES], f32)
            xt_bf = persist.tile([128, NJ * B_C], bf16)
            wp_sb = persist.tile([128, S * 128], bf16)

            # --- load plan: head alternates sync/scalar HWDGE rings so the
            # first chains' data lands ~2x faster; tail rides sync so the
            # scalar engine is free for store triggers from ~12us on.
            # Node0's w chunk is split so the first LDWEIGHTS waits on only
            # ~64KB; bias is deferred (first evac needs it only at ~14us). ---
            emitted_w = 0   # how many w chunks issued
            def emit_w_chunk(eng, split_first=False):
                nonlocal emitted_w
                if emitted_w < len(wchunks):
                    s0, n, _last = wchunks[emitted_w]
                    if split_first and n > 2:
                        eng.dma_start(out=wp_sb[:, s0 * 128:(s0 + 2) * 128],
                                      in_=wp_d[:, s0 * 128:(s0 + 2) * 128])
                        eng.dma_start(
                            out=wp_sb[:, (s0 + 2) * 128:(s0 + n) * 128],
                            in_=wp_d[:, (s0 + 2) * 128:(s0 + n) * 128])
                    else:
                        eng.dma_start(out=wp_sb[:, s0 * 128:(s0 + n) * 128],
                                      in_=wp_d[:, s0 * 128:(s0 + n) * 128])
                    emitted_w += 1

            def emit_x(node_idx, eng, per_block=False):
                cur = new_js[node_idx]
                if not cur:
                    return
                p0 = pos[cur[0]]
                n = len(cur)
                if per_block:
                    for t in range(n):
                        e = (nc.sync, nc.scalar)[t % 2]
                        e.dma_start(
                            out=xt_bf[:, (p0 + t) * B_C:(p0 + t + 1) * B_C],
                            in_=xt_d[:, (p0 + t) * B_C:(p0 + t + 1) * B_C])
                else:
                    eng.dma_start(
                        out=xt_bf[:, p0 * B_C:(p0 + n) * B_C],
                        in_=xt_d[:, p0 * B_C:(p0 + n) * B_C])

            # head: node0's w on scalar (split), its x blocks alternating
            emit_w_chunk(nc.scalar, split_first=True)
            emit_x(0, None, per_block=True)
            nc.scalar.dma_start(out=bias_sb[:], in_=bias_d[:])
            k = 1
            hx = 0
            while emitted_w < len(wchunks):
                _s0, _n, last = wchunks[emitted_w]
                emit_w_chunk(nc.sync)
                while k <= last:
                    if new_js[k]:
                        hx += 1
                    emit_x(k, nc.scalar if hx <= 2 else nc.sync)
                    k += 1

            # --- compute: per node, two interleaved 512-wide accumulation
            # chains into one 2-bank PSUM tile, single wide evac + store.
            # The last node runs its two half-chains SEQUENTIALLY so the
            # first half's evac+store overlaps the second half's matmuls,
            # leaving only half an evac+store exposed after the last MM. ---
            s0 = 0
            n_sched = len(sched)
            for kk, (i, js, _zero) in enumerate(sched):
                nj = len(js)
                ps = psump.tile([128, B_C], f32, tag="acc", name=f"acc_{i}")
                ot = outp.tile([128, B_C], bf16, tag="ot")
                H = B_C // 2
                if kk >= n_sched - 1:
                    for bt in range(2):
                        for idx, j in enumerate(js):
                            nc.tensor.matmul(
                                ps[:, bt * H:(bt + 1) * H],
                                wp_sb[:, (s0 + idx) * 128:(s0 + idx + 1) * 128],
                                xt_bf[:, pos[j] * B_C + bt * H:
                                      pos[j] * B_C + (bt + 1) * H],
                                start=(idx == 0),
                                stop=(idx == nj - 1),
                            )
                        nc.vector.tensor_scalar_add(
                            ot[:, bt * H:(bt + 1) * H],
                            ps[:, bt * H:(bt + 1) * H], bias_sb[:, i:i + 1])
                        nc.scalar.dma_start(
                            out=out_d[i * 128:(i + 1) * 128, bt * H:(bt + 1) * H],
                            in_=ot[:, bt * H:(bt + 1) * H])
                else:
                    for idx, j in enumerate(js):
                        w_ap = wp_sb[:, (s0 + idx) * 128:(s0 + idx + 1) * 128]
                        for bt in range(2):
                            nc.tensor.matmul(
                                ps[:, bt * H:(bt + 1) * H],
                                w_ap,
                                xt_bf[:, pos[j] * B_C + bt * H:
                                      pos[j] * B_C + (bt + 1) * H],
                                start=(idx == 0),
                                stop=(idx == nj - 1),
                            )
                    nc.vector.tensor_scalar_add(ot[:], ps[:],
                                                bias_sb[:, i:i + 1])
                    nc.scalar.dma_start(
                        out=out_d[i * 128:(i + 1) * 128, :], in_=ot[:])
                s0 += nj

    nc.compile()
    return nc


def _get_graph(adjacency):
    sched, jorder = _build_schedule(adjacency)
    key = (sched, jorder)
    if key not in _CACHE:
        _CACHE[key] = (_build_graph(sched, jorder), sched, jorder)
    return _CACHE[key]


def _pack_inputs(x, weight, bias, sched, jorder):
    """Build the 8 per-core input maps (host-side slicing/layout only)."""
    bf16 = ml_dtypes.bfloat16
    x = np.asarray(x, dtype=np.float32)
    weight = np.asarray(weight, dtype=np.float32)
    bias = np.asarray(bias, dtype=np.float32)

    flat = []  # (i, j, zero) in slot order
    for i, js, zero in sched:
        for j in js:
            flat.append((i, j, zero))
    S = len(flat)

    w5 = weight.reshape(NUM_NODES, 2, 128, NUM_NODES, IN_F)  # i, h, o, j, k
    w5t = w5.transpose(1, 4, 0, 3, 2)                        # h, k, i, j, o

    si = np.array([f[0] for f in flat])
    sj = np.array([f[1] for f in flat])
    szero = np.array([f[2] for f in flat])

    wp_h = []
    for h in range(2):
        wp = np.ascontiguousarray(w5t[h][:, si, sj, :])      # [128, S, 128]
        if szero.any():
            wp[:, szero, :] = 0.0
        wp_h.append(wp.reshape(128, S * 128).astype(bf16))

    bias3 = bias.reshape(NUM_NODES, 2, 128)
    bias_h = [np.ascontiguousarray(bias3[:, h, :].T) for h in range(2)]

    jord = list(jorder)
    in_maps = []
    xt_cache = {}
    for c in range(N_CORES):
        bq, h = divmod(c, 2)
        if bq not in xt_cache:
            # [128 k, NJ blocks (first-use order), 1024 batch]
            xq = x[bq * B_C:(bq + 1) * B_C][:, jord, :]      # [1024, NJ, 128]
            xt_cache[bq] = np.ascontiguousarray(
                xq.transpose(2, 1, 0).reshape(128, len(jord) * B_C)).astype(bf16)
        in_maps.append({
            "xt": xt_cache[bq],
            "wp": wp_h[h],
            "biasr": bias_h[h],
        })
    return in_maps


def _gather_output(results):
    y = np.empty((P_BATCH, B_C, NUM_NODES, 2, 128), dtype=np.float32)
    for c in range(N_CORES):
        bq, h = divmod(c, 2)
        oc = results[c]["out"].astype(np.float32).reshape(NUM_NODES, 128, B_C)
        y[bq, :, :, h, :] = oc.transpose(2, 0, 1)
    return y.reshape(BATCH, NUM_NODES, OUT_F)


def _ensure_axon_profile_hook():
    """Provide antenv.axon_hooks if the image lacks it (no-op otherwise).

    concourse.bass_utils imports antenv.axon_hooks on the trace path; some
    images miss the module, which would turn BASS_TRACE=1 into an
    ImportError. Registers the standard ctypes NTFF hook when possible.
    """
    try:
        import antenv.axon_hooks  # noqa: F401
        return
    except ImportError:
        pass
    try:
        import antenv
    except ImportError:
        return
    import contextlib
    import ctypes
    import sys
    import types

    hook = None
    try:
        lib = ctypes.CDLL("/opt/axon/libaxon_pjrt.so")
        if hasattr(lib, "axon_start_nrt_profile"):
            lib.axon_start_nrt_profile.argtypes = [
                ctypes.POINTER(ctypes.c_int64), ctypes.c_size_t]
            lib.axon_start_nrt_profile.restype = ctypes.c_int64
            lib.axon_stop_nrt_profile.argtypes = [ctypes.c_char_p]
            lib.axon_stop_nrt_profile.restype = ctypes.c_int64

            @contextlib.contextmanager
            def hook(output_dir, device_ids):
                import jax
                jax.devices()
                if device_ids:
                    ids = (ctypes.c_int64 * len(device_ids))(*device_ids)
                    rc = lib.axon_start_nrt_profile(ids, len(device_ids))
                else:
                    rc = lib.axon_start_nrt_profile(None, 0)
                if rc != 0:
                    raise RuntimeError(f"axon_start_nrt_profile rc={rc}")
                try:
                    yield
                finally:
                    lib.axon_stop_nrt_profile(str(output_dir).encode())
    except OSError:
        hook = None

    mod = types.ModuleType("antenv.axon_hooks")
    mod._hook = hook
    mod.get_axon_ntff_profile_hook = lambda: mod._hook

    def _set(h):
        mod._hook = h

    mod.set_axon_ntff_profile_hook = _set
    sys.modules["antenv.axon_hooks"] = mod
    antenv.axon_hooks = mod


def kernel(x, weight, bias, adjacency):
    from concourse.bass_utils import run_bass_kernel_spmd

    _ensure_axon_profile_hook()
    nc, sched, jorder = _get_graph(adjacency)
    in_maps = _pack_inputs(x, weight, bias, sched, jorder)

    kwargs = {}
    if os.environ.get("KERNEL_TRACE"):
        kwargs["trace"] = True
        tcores = os.environ.get("KERNEL_TRACE_CORES")
        if tcores:
            kwargs["trace_cores"] = [int(t) for t in tcores.split(",")]

    res = run_bass_kernel_spmd(nc, in_maps, core_ids=list(range(N_CORES)),
                               **kwargs)
    kernel.last_result = res
    return _gather_output(res.results)


kernel.last_result = None


# revision 17
# speedup vs baseline: 1.1509x; 1.1509x over previous
"""Trainium2 Bass kernel for AnatomicalMaskedLinear (block-masked dense layer).

Reference op:
    mask  = kron(adjacency, ones(256, 128))            # (21*256, 21*128)
    y     = x.reshape(B, 21*128) @ (weight*mask).T + bias
    out   = y.reshape(B, 21, 256)

Strategy (v2):
  * Only nonzero (256o x 128k) blocks are shipped/matmul'd (S=nnz blocks).
  * 8 cores = 4 batch quarters x 2 node-row halves; all cores share one
    SPMD graph (same adjacency-derived schedule), only data differs.
  * bf16 operands, 1024-wide moving operand: one matmul per (node, j)
    block into a 2-bank PSUM tile [128, 1024] -> 237 matmuls/core instead
    of 474 (less NX issue overhead, fewer LDWEIGHTS).
  * PE warm-up: HAM clock gate holds the PE at 1.2 GHz until ~3.4us of
    sustained activity. A memset + dummy-matmul burst issued before the
    first real data arrives flips the gate early so the real stream runs
    at 2.4 GHz almost immediately.
  * All loads ride the sync HWDGE ring in first-use order (x is packed
    host-side in first-use block order so bulk triggers are contiguous);
    the first node's blocks use per-block triggers so the first matmul
    fires as soon as ~0.5 MB has landed instead of waiting for a bulk
    transfer. Everything stays resident in SBUF (13 MB total).
  * PSUM evacuation (bias add + f32->bf16 cast, activation Identity with
    per-partition bias) AND the store trigger both run on the scalar
    engine: evac->store needs no cross-engine event, and output traffic
    halves (bf16). Host upcasts to f32.
  * Fewer DMA triggers + fewer cross-engine events shrink the end-of-NEFF
    event-semaphore drain ladder, which counts toward measured exec time.
"""

import os
import numpy as np
import ml_dtypes

NUM_NODES = 21
IN_F = 128
OUT_F = 256
BATCH = 4096
N_CORES = 8
P_BATCH = 4                      # batch ways
B_C = BATCH // P_BATCH           # 1024 batch rows per core
K_TOTAL = NUM_NODES * IN_F       # 2688
O_C = NUM_NODES * 128            # 2688 out rows per core (half of each node)
N_WARM = 6                       # 512-wide dummy matmuls (HAM clock gate)
N_WARM_MID = 3                   # warm-keeper dummies after each early chain
FP8_PAIRS = 40                   # DoubleRow fp8 pairs (2 edges each)
FP8_MIN_SCHED = 8                # only nodes at sched idx >= this use fp8
W_SCALE = 512.0                  # bf16/fp8 weights pre-scaled; evac multiplies 1/512

_CACHE = {}                      # schedule key -> (nc, sched, jorder)


def _node_order(active):
    """Greedy: minimize newly-required x blocks at each step."""
    loaded = set()
    remaining = set(range(NUM_NODES))
    order = []
    while remaining:
        nxt = min(remaining,
                  key=lambda i: (len(set(active[i]) - loaded), len(active[i]), i))
        order.append(nxt)
        loaded |= set(active[nxt])
        remaining.remove(nxt)
    return order


def _build_schedule(adjacency):
    """[(i, units, zero_pad)] in greedy node order; >=1 unit per node.

    units: ('b', j) = one bf16 block matmul; ('d', pa) = fp8 DoubleRow pair
    covering x blocks jorder[pa], jorder[pa+1]. DR units come first within a
    node; bf16 units keep already-loaded x blocks before newly-required ones.
    """
    A = np.asarray(adjacency) != 0
    active = {i: [int(j) for j in np.where(A[i])[0]] for i in range(NUM_NODES)}
    order = _node_order(active)
    jorder = []
    seen = set()
    for i in order:
        for j in active[i]:
            if j not in seen:
                seen.add(j)
                jorder.append(j)
    if not jorder:
        jorder = [0]
    pos = {j: p for p, j in enumerate(jorder)}
    sched = []
    loaded = set()
    pair_budget = FP8_PAIRS
    for kk, i in enumerate(order):
        js = active[i]
        if not js:
            sched.append((i, (('b', jorder[0]),), True))
            continue
        olds = sorted((j for j in js if j in loaded), key=lambda j: pos[j])
        news = sorted((j for j in js if j not in loaded), key=lambda j: pos[j])
        loaded |= set(js)
        rest = olds + news
        units = []
        if kk >= FP8_MIN_SCHED and pair_budget > 0:
            ps_ = sorted(pos[j] for j in js)
            pset = set(ps_)
            used = set()
            for a in ps_:
                if pair_budget <= 0 or a in used:
                    continue
                if a + 1 in pset and a + 1 not in used:
                    units.append(('d', a))
                    used.add(a)
                    used.add(a + 1)
                    pair_budget -= 1
            paired_js = {jorder[a] for u, a in units} | {jorder[a + 1] for u, a in units}
            rest = [j for j in rest if j not in paired_js]
        units.extend(('b', j) for j in rest)
        sched.append((i, tuple(units), False))
    return tuple(sched), tuple(jorder)


def _w_chunks(sched):
    """Group bf16 slots into w DMA chunks: per-node for the first few nodes,
    then ~1MB merged chunks."""
    sizes = [sum(1 for u in units if u[0] == 'b') for _, units, _ in sched]
    chunks = []  # (start_slot, n_slots, last_sched_idx)
    k = 0
    s0 = 0
    while k < len(sched):
        if k < 7:
            n = sizes[k]
            last = k
            k += 1
        else:
            n = 0
            last = k
            while k < len(sched) and (n == 0 or n + sizes[k] <= 34):
                n += sizes[k]
                last = k
                k += 1
        chunks.append((s0, n, last))
        s0 += n
    return chunks


def _build_graph(sched, jorder):
    import concourse.tile as tile
    from concourse import bacc, mybir

    S = sum(1 for _, units, _ in sched for u in units if u[0] == 'b')
    P = sum(1 for _, units, _ in sched for u in units if u[0] == 'd')
    f32 = mybir.dt.float32
    bf16 = mybir.dt.bfloat16
    f8e4 = mybir.dt.float8e4
    pos = {j: p for p, j in enumerate(jorder)}
    NJ = len(jorder)

    nc = bacc.Bacc("TRN2", target_bir_lowering=False, debug=False,
                   num_devices=N_CORES)

    xt_d = nc.declare_dram_parameter("xt", [128, NJ * B_C], bf16, isOutput=False)
    wp_d = nc.declare_dram_parameter("wp", [128, S * 128], bf16, isOutput=False)
    bias_d = nc.declare_dram_parameter("biasr", [128, NUM_NODES], f32,
                                       isOutput=False)
    out_d = nc.declare_dram_parameter("out", [O_C, B_C], bf16, isOutput=True)
    if P:
        x8_d = nc.declare_dram_parameter("x8", [128, NJ, B_C], f8e4,
                                         isOutput=False)
        wp8_d = nc.declare_dram_parameter("wp8", [128, 2 * P, 128], f8e4,
                                          isOutput=False)

    # per-node newly needed x blocks (contiguous in jorder by construction)
    new_js = []
    seen = set()
    for i, units, zero in sched:
        js = set()
        for u in units:
            if u[0] == 'b':
                js.add(u[1])
            else:
                js.add(jorder[u[1]])
                js.add(jorder[u[1] + 1])
        cur = [] if zero else sorted((j for j in js if j not in seen),
                                     key=lambda j: pos[j])
        seen |= set(cur)
        new_js.append(cur)

    wchunks = _w_chunks(sched)

    with tile.TileContext(nc) as tc:
        with (
            tc.tile_pool(name="const", bufs=1) as constp,
            tc.tile_pool(name="persist", bufs=1) as persist,
            tc.tile_pool(name="psum", bufs=3, space="PSUM") as psump,
            tc.tile_pool(name="wps", bufs=1, space="PSUM") as warmpp,
            tc.tile_pool(name="outp", bufs=8) as outp,
        ):
            # --- PE warm-up: memset a small bf16 tile, then dummy matmuls
            # (HAM clock gate: PE runs at 1.2 GHz until ~3.4us of sustained
            # activity; dummies must span the whole DMA wait or the gate
            # re-throttles during the idle gap) ---
            warm_sb = constp.tile([128, 512], bf16)
            nc.gpsimd.memset(warm_sb[:], 0.0)
            warm_ps = warmpp.tile([128, 512], f32)
            for t in range(N_WARM):
                nc.tensor.matmul(warm_ps[:], warm_sb[:, 0:128],
                                 warm_sb[:], start=True, stop=True)

            def warm_keep(n):
                for _t in range(n):
                    nc.tensor.matmul(warm_ps[:], warm_sb[:, 0:128],
                                     warm_sb[:], start=True, stop=True)

            bias_sb = constp.tile([128, NUM_NODES], f32)
            xt_bf = persist.tile([128, NJ * B_C], bf16)
            wp_sb = persist.tile([128, S * 128], bf16)
            if P:
                x8_sb = persist.tile([128, NJ, B_C], f8e4)
                wp8_sb = persist.tile([128, 2 * P, 128], f8e4)

            # --- load plan: head alternates sync/scalar HWDGE rings so the
            # first chains' data lands ~2x faster; tail rides sync so the
            # scalar engine is free for store triggers from ~12us on.
            # Node0's w chunk is split so the first LDWEIGHTS waits on only
            # ~64KB; bias is deferred (first evac needs it only at ~14us). ---
            emitted_w = 0   # how many w chunks issued
            def emit_w_chunk(eng, split_first=False):
                nonlocal emitted_w
                if emitted_w < len(wchunks):
                    s0, n, _last = wchunks[emitted_w]
                    if split_first and n > 2:
                        eng.dma_start(out=wp_sb[:, s0 * 128:(s0 + 2) * 128],
                                      in_=wp_d[:, s0 * 128:(s0 + 2) * 128])
                        eng.dma_start(
                            out=wp_sb[:, (s0 + 2) * 128:(s0 + n) * 128],
                            in_=wp_d[:, (s0 + 2) * 128:(s0 + n) * 128])
                    else:
                        eng.dma_start(out=wp_sb[:, s0 * 128:(s0 + n) * 128],
                                      in_=wp_d[:, s0 * 128:(s0 + n) * 128])
                    emitted_w += 1

            def emit_x(node_idx, eng, per_block=False):
                cur = new_js[node_idx]
                if not cur:
                    return
                p0 = pos[cur[0]]
                n = len(cur)
                if per_block:
                    for t in range(n):
                        e = (nc.sync, nc.scalar)[t % 2]
                        e.dma_start(
                            out=xt_bf[:, (p0 + t) * B_C:(p0 + t + 1) * B_C],
                            in_=xt_d[:, (p0 + t) * B_C:(p0 + t + 1) * B_C])
                else:
                    eng.dma_start(
                        out=xt_bf[:, p0 * B_C:(p0 + n) * B_C],
                        in_=xt_d[:, p0 * B_C:(p0 + n) * B_C])

            # head: node0's w on scalar (split), its x blocks alternating
            emit_w_chunk(nc.scalar, split_first=True)
            emit_x(0, None, per_block=True)
            nc.scalar.dma_start(out=bias_sb[:], in_=bias_d[:])
            k = 1
            hx = 0
            fp8_emitted = False
            while emitted_w < len(wchunks):
                _s0, _n, last = wchunks[emitted_w]
                emit_w_chunk(nc.sync)
                while k <= last:
                    if new_js[k]:
                        hx += 1
                    emit_x(k, nc.scalar if hx <= 2 else nc.sync)
                    k += 1
                if P and not fp8_emitted and last >= 7:
                    nc.sync.dma_start(out=wp8_sb[:], in_=wp8_d[:])
                    nc.sync.dma_start(out=x8_sb[:], in_=x8_d[:])
                    fp8_emitted = True

            # --- compute: per node, two interleaved 512-wide accumulation
            # chains into one 2-bank PSUM tile (fp8 DoubleRow pairs first,
            # then bf16 blocks), single wide evac + store. Weights are
            # pre-scaled by W_SCALE; the evac multiplies by 1/W_SCALE before
            # adding the bias. The last node runs its two half-chains
            # SEQUENTIALLY so only half an evac+store trails the last MM.
            # Warm-keeper dummies after early chains hold the HAM clock gate
            # open through the delivery-bound head. ---
            inv_scale = 1.0 / W_SCALE
            mult_op = mybir.AluOpType.mult
            add_op = mybir.AluOpType.add
            s0 = 0
            q0 = 0
            n_sched = len(sched)

            def emit_mm(ps_ap, unit, slot, bt, start, stop):
                H = B_C // 2
                if unit[0] == 'b':
                    j = unit[1]
                    nc.tensor.matmul(
                        ps_ap[:, bt * H:(bt + 1) * H],
                        wp_sb[:, slot * 128:(slot + 1) * 128],
                        xt_bf[:, pos[j] * B_C + bt * H:
                              pos[j] * B_C + (bt + 1) * H],
                        start=start, stop=stop)
                else:
                    pa = unit[1]
                    nc.tensor.matmul(
                        ps_ap[:, bt * H:(bt + 1) * H],
                        wp8_sb[:, 2 * slot:2 * slot + 2, :],
                        x8_sb[:, pa:pa + 2, bt * H:(bt + 1) * H],
                        start=start, stop=stop,
                        perf_mode=mybir.MatmulPerfMode.DoubleRow)

            for kk, (i, units, _zero) in enumerate(sched):
                nu = len(units)
                nb = sum(1 for u in units if u[0] == 'b')
                nd = nu - nb
                ps = psump.tile([128, B_C], f32, tag="acc", name=f"acc_{i}")
                ot = outp.tile([128, B_C], bf16, tag="ot")
                H = B_C // 2
                if kk >= n_sched - 1:
                    for bt in range(2):
                        sq, ss = q0, s0
                        for idx, u in enumerate(units):
                            slot = sq if u[0] == 'd' else ss
                            if u[0] == 'd':
                                sq += 1
                            else:
                                ss += 1
                            emit_mm(ps, u, slot, bt, idx == 0, idx == nu - 1)
                        nc.vector.tensor_scalar(
                            ot[:, bt * H:(bt + 1) * H],
                            ps[:, bt * H:(bt + 1) * H],
                            inv_scale, bias_sb[:, i:i + 1], mult_op, add_op)
                        nc.scalar.dma_start(
                            out=out_d[i * 128:(i + 1) * 128, bt * H:(bt + 1) * H],
                            in_=ot[:, bt * H:(bt + 1) * H])
                    q0 += nd
                    s0 += nb
                else:
                    sq, ss = q0, s0
                    for idx, u in enumerate(units):
                        slot = sq if u[0] == 'd' else ss
                        if u[0] == 'd':
                            sq += 1
                        else:
                            ss += 1
                        for bt in range(2):
                            emit_mm(ps, u, slot, bt, idx == 0, idx == nu - 1)
                    q0 += nd
                    s0 += nb
                    nc.vector.tensor_scalar(ot[:], ps[:], inv_scale,
                                            bias_sb[:, i:i + 1], mult_op, add_op)
                    nc.scalar.dma_start(
                        out=out_d[i * 128:(i + 1) * 128, :], in_=ot[:])
                if kk < 4:
                    warm_keep(N_WARM_MID)

    nc.compile()
    return nc


def _get_graph(adjacency):
    sched, jorder = _build_schedule(adjacency)
    key = (sched, jorder)
    if key not in _CACHE:
        _CACHE[key] = (_build_graph(sched, jorder), sched, jorder)
    return _CACHE[key]


def _pack_inputs(x, weight, bias, sched, jorder):
    """Build the 8 per-core input maps (host-side slicing/layout only)."""
    bf16 = ml_dtypes.bfloat16
    x = np.asarray(x, dtype=np.float32)
    weight = np.asarray(weight, dtype=np.float32)
    bias = np.asarray(bias, dtype=np.float32)

    flat = []  # (i, j, zero) in slot order
    for i, js, zero in sched:
        for j in js:
            flat.append((i, j, zero))
    S = len(flat)

    w5 = weight.reshape(NUM_NODES, 2, 128, NUM_NODES, IN_F)  # i, h, o, j, k
    w5t = w5.transpose(1, 4, 0, 3, 2)                        # h, k, i, j, o

    si = np.array([f[0] for f in flat])
    sj = np.array([f[1] for f in flat])
    szero = np.array([f[2] for f in flat])

    wp_h = []
    for h in range(2):
        wp = np.ascontiguousarray(w5t[h][:, si, sj, :])      # [128, S, 128]
        if szero.any():
            wp[:, szero, :] = 0.0
        wp_h.append(wp.reshape(128, S * 128).astype(bf16))

    bias3 = bias.reshape(NUM_NODES, 2, 128)
    bias_h = [np.ascontiguousarray(bias3[:, h, :].T) for h in range(2)]

    jord = list(jorder)
    in_maps = []
    xt_cache = {}
    for c in range(N_CORES):
        bq, h = divmod(c, 2)
        if bq not in xt_cache:
            # [128 k, NJ blocks (first-use order), 1024 batch]
            xq = x[bq * B_C:(bq + 1) * B_C][:, jord, :]      # [1024, NJ, 128]
            xt_cache[bq] = np.ascontiguousarray(
                xq.transpose(2, 1, 0).reshape(128, len(jord) * B_C)).astype(bf16)
        in_maps.append({
            "xt": xt_cache[bq],
            "wp": wp_h[h],
            "biasr": bias_h[h],
        })
    return in_maps


def _gather_output(results):
    y = np.empty((P_BATCH, B_C, NUM_NODES, 2, 128), dtype=np.float32)
    for c in range(N_CORES):
        bq, h = divmod(c, 2)
        oc = results[c]["out"].astype(np.float32).reshape(NUM_NODES, 128, B_C)
        y[bq, :, :, h, :] = oc.transpose(2, 0, 1)
    return y.reshape(BATCH, NUM_NODES, OUT_F)


def _ensure_axon_profile_hook():
    """Provide antenv.axon_hooks if the image lacks it (no-op otherwise).

    concourse.bass_utils imports antenv.axon_hooks on the trace path; some
    images miss the module, which would turn BASS_TRACE=1 into an
    ImportError. Registers the standard ctypes NTFF hook when possible.
    """
    try:
        import antenv.axon_hooks  # noqa: F401
        return
    except ImportError:
        pass
    try:
        import antenv
    except ImportError:
        return
    import contextlib
    import ctypes
    import sys
    import types

    hook = None
    try:
        lib = ctypes.CDLL("/opt/axon/libaxon_pjrt.so")
        if hasattr(lib, "axon_start_nrt_profile"):
            lib.axon_start_nrt_profile.argtypes = [
                ctypes.POINTER(ctypes.c_int64), ctypes.c_size_t]
            lib.axon_start_nrt_profile.restype = ctypes.c_int64
            lib.axon_stop_nrt_profile.argtypes = [ctypes.c_char_p]
            lib.axon_stop_nrt_profile.restype = ctypes.c_int64

            @contextlib.contextmanager
            def hook(output_dir, device_ids):
                import jax
                jax.devices()
                if device_ids:
                    ids = (ctypes.c_int64 * len(device_ids))(*device_ids)
                    rc = lib.axon_start_nrt_profile(ids, len(device_ids))
                else:
                    rc = lib.axon_start_nrt_profile(None, 0)
                if rc != 0:
                    raise RuntimeError(f"axon_start_nrt_profile rc={rc}")
                try:
                    yield
                finally:
                    lib.axon_stop_nrt_profile(str(output_dir).encode())
    except OSError:
        hook = None

    mod = types.ModuleType("antenv.axon_hooks")
    mod._hook = hook
    mod.get_axon_ntff_profile_hook = lambda: mod._hook

    def _set(h):
        mod._hook = h

    mod.set_axon_ntff_profile_hook = _set
    sys.modules["antenv.axon_hooks"] = mod
    antenv.axon_hooks = mod


def kernel(x, weight, bias, adjacency):
    from concourse.bass_utils import run_bass_kernel_spmd

    _ensure_axon_profile_hook()
    nc, sched, jorder = _get_graph(adjacency)
    in_maps = _pack_inputs(x, weight, bias, sched, jorder)

    kwargs = {}
    if os.environ.get("KERNEL_TRACE"):
        kwargs["trace"] = True
        tcores = os.environ.get("KERNEL_TRACE_CORES")
        if tcores:
            kwargs["trace_cores"] = [int(t) for t in tcores.split(",")]

    res = run_bass_kernel_spmd(nc, in_maps, core_ids=list(range(N_CORES)),
                               **kwargs)
    kernel.last_result = res
    return _gather_output(res.results)


kernel.last_result = None
